# revision 35
# baseline (speedup 1.0000x reference)
"""Trainium2 Bass kernel for nn_ADDSLoss (retrieval_knn).

Math:
  pred = points @ R_p^T + t_p ; gt = points @ R_g^T + t_g          (per batch)
  out  = mean_{b,m} min_n ||pred[b,m] - gt[b,n]||

Device formulation (per batch):
  d2[m,n] = pn[m] + gn[n] - 2*pred[m].gt[n] = sum_k L[k,m]*R[k,n]   (K=5)
  with L = W_L @ F, R = W_R @ F, where F is the degree-2 polynomial
  feature map of the raw model points, built on device:
    F rows = [x, y, z, 1, x^2, y^2, z^2, x*y, y*z, z*x]   [10, M]
  and W_L/W_R are tiny per-batch 5x10 matrices (host, O(B) work):
    L rows = [pred_x, pred_y, pred_z, 1, |pred|^2]
    R rows = [-2 gt_x, -2 gt_y, -2 gt_z, |gt|^2, 1]
  min_n sqrt(max(d2,0)) == sqrt(max(min_n d2, 0)), so sqrt happens on
  M values per batch, not M^2.

Sharding: data-parallel over B across the 8 cores (2 batches/core). Each
core returns its partial sum of min-distances; the final mean is the sum
of the 8 partials / (B*M) on host (a trivial 8-float reduction).

Device pipeline per core:
  phase 0: DMA points^T + W matrices; build F via two DVE multiplies.
  phase 1: transform matmuls (fp32, K=10) -> PSUM -> cast-evac to fp16
           L/R operand tensors [5, M].
  phase 2: per (m-tile, batch): 8 fp16 matmuls (K=5, N=512) -> d2 tiles
           in PSUM; two chained vector.tensor_tensor_reduce(min) ops
           min-reduce 4096 gt candidates straight out of PSUM into a
           [128,1] running min per m-tile.
  phase 3: relu -> sqrt -> free-axis sum -> partition all-reduce -> DMA out.
"""

import functools

import numpy as np

B, M = 16, 4096
NCORES = 8
BPC = B // NCORES        # batches per core
NCH = M // 512           # 8 rhs chunks of 512
NMT = M // 128           # 32 m-tiles of 128
NSIDES = 2 * BPC         # L0, R0, L1, R1
BIG = 3.0e38             # running-min init (finite to avoid inf handling)


def _quad_row(Rm, t):
    """Feature-space weights of ||R p + t||^2 over
    [x, y, z, 1, x2, y2, z2, xy, yz, zx]."""
    A = Rm.T @ Rm
    row = np.zeros(10, np.float64)
    row[0:3] = 2.0 * (Rm.T @ t)
    row[3] = float(t @ t)
    row[4:7] = np.diag(A)
    row[7] = 2.0 * A[0, 1]
    row[8] = 2.0 * A[1, 2]
    row[9] = 2.0 * A[0, 2]
    return row


def _w_pred(Rp, tp):
    W = np.zeros((5, 10), np.float64)
    W[0:3, 0:3] = Rp
    W[0:3, 3] = tp
    W[3, 3] = 1.0
    W[4] = _quad_row(Rp, tp)
    return W


def _w_gt(Rg, tg):
    W = np.zeros((5, 10), np.float64)
    W[0:3, 0:3] = -2.0 * Rg
    W[0:3, 3] = -2.0 * tg
    W[3] = _quad_row(Rg, tg)
    W[4, 3] = 1.0
    return W


@functools.lru_cache(maxsize=1)
def _build_graph():
    import concourse.mybir as mybir
    import concourse.tile as tile
    from concourse import bacc

    f32 = mybir.dt.float32
    f16 = mybir.dt.bfloat16  # 16-bit operand dtype for the main matmuls
    AL = mybir.AluOpType

    nc = bacc.Bacc(
        "TRN2", target_bir_lowering=False, debug=False, num_devices=NCORES
    )
    f32r = mybir.dt.float32r
    feat_d = nc.declare_dram_parameter("feat", [10, M], f32r, isOutput=False)
    wm_d = nc.declare_dram_parameter("wmat", [10, 128], f32r, isOutput=False)
    out_d = nc.declare_dram_parameter("out", [1, 1], f32, isOutput=True)

    with tile.TileContext(nc) as tc:
        with (
            tc.tile_pool(name="constp", bufs=1) as constp,
            tc.tile_pool(name="accp", bufs=6) as accp,
        ):
            F = constp.tile([10, M], f32r)
            wm = constp.tile([10, 128], f32r)
            sbig = constp.tile([128, M], f16)
            mind2 = constp.tile([128, NMT * BPC], f32)
            dist = constp.tile([128, NMT * BPC], f32)
            colsum = constp.tile([128, 1], f32)
            dummy = constp.tile([128, 1], f32)
            # per-side operand tensors replicated at partition offsets
            # {0,32,64,96} so four K=5 matmuls can run concurrently in the
            # PE array via tile_position row-packing; split into halves so
            # phase 2 can start as soon as the first half is transformed
            lrh = [
                [
                    constp.tile([128, M // 2], f16, name=f"lr{s}h{h}")
                    for h in range(2)
                ]
                for s in range(NSIDES)
            ]

            # ---- phase 0: inputs ----
            nc.sync.dma_start(wm[:], wm_d[:])
            # touch Sqrt now so its ACT table set loads during the prelude
            # (Copy lives in every set, so no second table switch later);
            # scale=0/bias=1 keeps the probe input in sqrt's valid range
            nc.scalar.activation(
                dummy[0:10, 0:1],
                wm[:, 0:1],
                mybir.ActivationFunctionType.Sqrt,
                bias=1.0,
                scale=0.0,
            )
            # chunked so the first transform matmul starts after 1/8 of F
            for q in range(NCH):
                nc.sync.dma_start(
                    F[:, q * 512 : (q + 1) * 512],
                    feat_d[:, q * 512 : (q + 1) * 512],
                )

            # ---- phase 1: L/R transforms (fp32 exact) ----
            # All four 5x10 transform matrices are stacked into one
            # [10, 128] lhsT (side s at columns 32s..32s+5): the fp32
            # matmul cost is per-row, independent of M, so one stacked
            # matmul per chunk does the work of four.  The [128, 512]
            # outputs are cast-evacuated to bf16 and the four 5-row
            # blocks DMA'd to partition 0 where the main matmuls expect
            # their K operands.
            # per-chunk evac + replica DMAs so phase 2 can start as soon as
            # the first half of the transforms lands
            with tc.tile_pool(name="pst", bufs=4, space="PSUM") as pst:
                for cc in range(NCH):
                    h, hc = cc // (NCH // 2), cc % (NCH // 2)
                    tp = pst.tile([128, 512], f32, tag="tp")
                    nc.tensor.matmul(tp[:], wm[:], F[:, cc * 512 : (cc + 1) * 512])
                    csl = slice(cc * 512, (cc + 1) * 512)
                    if cc % 2 == 0:
                        nc.scalar.copy(sbig[:, csl], tp[:])
                    else:
                        nc.vector.tensor_copy(sbig[:, csl], tp[:])
                    if hc == NCH // 2 - 1:
                        # replicate this half's four 5-row side blocks to the
                        # four 32-row offsets (half-granularity: DMA issue
                        # cost is ~700ns each, so keep the count low)
                        hof = h * (M // 2)
                        for s in range(NSIDES):
                            for i in range(4):
                                nc.sync.dma_start(
                                    lrh[s][h][32 * i : 32 * i + 5, :],
                                    sbig[32 * s : 32 * s + 5,
                                         hof : hof + M // 2],
                                )

            # ---- phase 2: pairwise d2 + min-reduce ----
            # HW allows only ONE PSUM operand per DVE op (and the custom
            # tensor_tensor_reduce ISA op is unavailable in this runtime),
            # so the d2 drain is split across standard ops: DVE
            # tensor_reduce(min) eats XDVE cols straight from PSUM (1x);
            # ScalarE evacuates the rest to fp16 SBUF where a 3-level
            # tensor_tensor(min) tree runs in the DVE 2x_1P mode, then a
            # final 1x reduce.  XDVE chosen so DVE and ACT are balanced.
            XDVE = 640
            E = 4096 - XDVE  # evacuated columns

            def emit_tree(eb, acc, col):
                l1 = ebp.tile([128, E // 2], f16, tag="l1")
                l2 = ebp.tile([128, E // 4], f16, tag="l2")
                l3 = ebp.tile([128, E // 8], f16, tag="l3")
                accb = accp.tile([128, 1], f32, tag="accb")
                nc.vector.tensor_tensor(
                    l1[:], eb[:, 0 : E // 2], eb[:, E // 2 : E], AL.min
                )
                nc.vector.tensor_tensor(
                    l2[:], l1[:, 0 : E // 4], l1[:, E // 4 : E // 2], AL.min
                )
                nc.vector.tensor_tensor(
                    l3[:], l2[:, 0 : E // 8], l2[:, E // 8 : E // 4], AL.min
                )
                nc.vector.tensor_reduce(
                    accb[:], l3[:], axis=mybir.AxisListType.X, op=AL.min
                )
                nc.vector.tensor_tensor(
                    mind2[:, col : col + 1], acc[:], accb[:], AL.min
                )

            # the fp16 tree of unit i is emitted during unit i+1 so the DVE
            # never stalls waiting for the ScalarE evacuation of its inputs
            with tc.tile_pool(name="psm", bufs=2, space="PSUM") as psm, \
                 tc.tile_pool(name="ebp", bufs=4) as ebp:
                pending = None
                for b in range(BPC):
                    for mt in range(NMT):
                        lb = lrh[2 * b][mt // (NMT // 2)]
                        rbh0 = lrh[2 * b + 1][0]
                        rbh1 = lrh[2 * b + 1][1]
                        mt0 = (mt % (NMT // 2)) * 128
                        mt1 = mt0 + 128
                        col = mt * BPC + b
                        acc = accp.tile([128, 1], f32, tag="acc")
                        eb = ebp.tile([128, E], f16, tag="eb")
                        pa = psm.tile([128, 2048], f32, tag="ps")
                        for c in range(4):
                            nc.tensor.matmul(
                                pa[:, c * 512 : (c + 1) * 512],
                                lb[32 * c : 32 * c + 5, mt0:mt1],
                                rbh0[32 * c : 32 * c + 5, c * 512 : (c + 1) * 512],
                                tile_position=(32 * c, 0),
                            )
                        nc.vector.tensor_reduce(
                            acc[:],
                            pa[:, 0:XDVE],
                            axis=mybir.AxisListType.X,
                            op=AL.min,
                        )
                        nc.scalar.copy(eb[:, 0 : 2048 - XDVE], pa[:, XDVE:2048])
                        pb = psm.tile([128, 2048], f32, tag="ps")
                        for c in range(4):
                            nc.tensor.matmul(
                                pb[:, c * 512 : (c + 1) * 512],
                                lb[32 * c : 32 * c + 5, mt0:mt1],
                                rbh1[32 * c : 32 * c + 5, c * 512 : (c + 1) * 512],
                                tile_position=(32 * c, 0),
                            )
                        nc.scalar.copy(eb[:, 2048 - XDVE : E], pb[:])
                        if pending is not None:
                            emit_tree(*pending)
                        pending = (eb, acc, col)
                emit_tree(*pending)

            # ---- phase 3: sqrt + reductions ----
            from concourse.bass_isa import ReduceOp

            nc.vector.tensor_scalar_max(dist[:], mind2[:], 0.0)
            nc.scalar.sqrt(mind2[:], dist[:])
            nc.vector.tensor_reduce(
                colsum[:], mind2[:], axis=mybir.AxisListType.X, op=AL.add
            )
            nc.gpsimd.partition_all_reduce(colsum[:], colsum[:], 128, ReduceOp.add)
            nc.sync.dma_start(out_d[:], colsum[0:1, :])

    nc.compile()
    return nc


def _host_inputs(pred_R, pred_t, gt_R, gt_t, model_points):
    # degree-2 polynomial feature map of the shared model points
    # (layout/feature prep of the constant cloud; all batch-dependent math
    # — transforms, distances, mins — runs on device)
    p = np.asarray(model_points, np.float32).T.astype(np.float32)  # [3, M]
    x, y, z = p[0], p[1], p[2]
    feat = np.stack(
        [x, y, z, np.ones_like(x), x * x, y * y, z * z, x * y, y * z, z * x]
    ).astype(np.float32)  # [10, M]
    in_maps = []
    for c in range(NCORES):
        wmat = np.zeros((10, 128), np.float32)
        for i in range(BPC):
            b = c * BPC + i
            wl = _w_pred(np.float64(pred_R[b]), np.float64(pred_t[b])).T
            wr = _w_gt(np.float64(gt_R[b]), np.float64(gt_t[b])).T
            wmat[:, 32 * (2 * i) : 32 * (2 * i) + 5] = wl
            wmat[:, 32 * (2 * i + 1) : 32 * (2 * i + 1) + 5] = wr
        in_maps.append({"feat": feat, "wmat": wmat})
    return in_maps


LAST_RESULT = None


def kernel(pred_R, pred_t, gt_R, gt_t, model_points):
    global LAST_RESULT
    from concourse.bass_utils import run_bass_kernel_spmd

    nc = _build_graph()
    in_maps = _host_inputs(pred_R, pred_t, gt_R, gt_t, model_points)
    res = run_bass_kernel_spmd(nc, in_maps, list(range(NCORES)))
    LAST_RESULT = res
    total = sum(float(res.results[c]["out"][0, 0]) for c in range(NCORES))
    return np.float32(total / (B * M))


# revision 38
# speedup vs baseline: 1.0061x; 1.0061x over previous
"""Trainium2 Bass kernel for nn_ADDSLoss (retrieval_knn).

Math:
  pred = points @ R_p^T + t_p ; gt = points @ R_g^T + t_g          (per batch)
  out  = mean_{b,m} min_n ||pred[b,m] - gt[b,n]||

Device formulation (per batch):
  d2[m,n] = pn[m] + gn[n] - 2*pred[m].gt[n] = sum_k L[k,m]*R[k,n]   (K=5)
  with L = W_L @ F, R = W_R @ F, where F is the degree-2 polynomial
  feature map of the raw model points, built on device:
    F rows = [x, y, z, 1, x^2, y^2, z^2, x*y, y*z, z*x]   [10, M]
  and W_L/W_R are tiny per-batch 5x10 matrices (host, O(B) work):
    L rows = [pred_x, pred_y, pred_z, 1, |pred|^2]
    R rows = [-2 gt_x, -2 gt_y, -2 gt_z, |gt|^2, 1]
  min_n sqrt(max(d2,0)) == sqrt(max(min_n d2, 0)), so sqrt happens on
  M values per batch, not M^2.

Sharding: data-parallel over B across the 8 cores (2 batches/core). Each
core returns its partial sum of min-distances; the final mean is the sum
of the 8 partials / (B*M) on host (a trivial 8-float reduction).

Device pipeline per core:
  phase 0: DMA points^T + W matrices; build F via two DVE multiplies.
  phase 1: transform matmuls (fp32, K=10) -> PSUM -> cast-evac to fp16
           L/R operand tensors [5, M].
  phase 2: per (m-tile, batch): 8 fp16 matmuls (K=5, N=512) -> d2 tiles
           in PSUM; two chained vector.tensor_tensor_reduce(min) ops
           min-reduce 4096 gt candidates straight out of PSUM into a
           [128,1] running min per m-tile.
  phase 3: relu -> sqrt -> free-axis sum -> partition all-reduce -> DMA out.
"""

import functools

import numpy as np

B, M = 16, 4096
NCORES = 8
BPC = B // NCORES        # batches per core
NCH = M // 512           # 8 rhs chunks of 512
NMT = M // 128           # 32 m-tiles of 128
NSIDES = 2 * BPC         # L0, R0, L1, R1
BIG = 3.0e38             # running-min init (finite to avoid inf handling)


def _quad_row(Rm, t):
    """Feature-space weights of ||R p + t||^2 over
    [x, y, z, 1, x2, y2, z2, xy, yz, zx]."""
    A = Rm.T @ Rm
    row = np.zeros(10, np.float64)
    row[0:3] = 2.0 * (Rm.T @ t)
    row[3] = float(t @ t)
    row[4:7] = np.diag(A)
    row[7] = 2.0 * A[0, 1]
    row[8] = 2.0 * A[1, 2]
    row[9] = 2.0 * A[0, 2]
    return row


def _w_pred(Rp, tp):
    W = np.zeros((5, 10), np.float64)
    W[0:3, 0:3] = Rp
    W[0:3, 3] = tp
    W[3, 3] = 1.0
    W[4] = _quad_row(Rp, tp)
    return W


def _w_gt(Rg, tg):
    W = np.zeros((5, 10), np.float64)
    W[0:3, 0:3] = -2.0 * Rg
    W[0:3, 3] = -2.0 * tg
    W[3] = _quad_row(Rg, tg)
    W[4, 3] = 1.0
    return W


@functools.lru_cache(maxsize=1)
def _build_graph():
    import concourse.mybir as mybir
    import concourse.tile as tile
    from concourse import bacc

    f32 = mybir.dt.float32
    f16 = mybir.dt.bfloat16  # 16-bit operand dtype for the main matmuls
    AL = mybir.AluOpType

    nc = bacc.Bacc(
        "TRN2", target_bir_lowering=False, debug=False, num_devices=NCORES
    )
    f32r = mybir.dt.float32r
    feat_d = nc.declare_dram_parameter("feat", [10, M], f32r, isOutput=False)
    wm_d = nc.declare_dram_parameter("wmat", [10, 128], f32r, isOutput=False)
    out_d = nc.declare_dram_parameter("out", [1, 1], f32, isOutput=True)

    with tile.TileContext(nc) as tc:
        with (
            tc.tile_pool(name="constp", bufs=1) as constp,
            tc.tile_pool(name="accp", bufs=6) as accp,
        ):
            F = constp.tile([10, M], f32r)
            wm = constp.tile([10, 128], f32r)
            sbig = constp.tile([128, M], f16)
            mind2 = constp.tile([128, NMT * BPC], f32)
            dist = constp.tile([128, NMT * BPC], f32)
            colsum = constp.tile([128, 1], f32)
            dummy = constp.tile([128, 1], f32)
            # per-side operand tensors replicated at partition offsets
            # {0,32,64,96} so four K=5 matmuls can run concurrently in the
            # PE array via tile_position row-packing; split into halves so
            # phase 2 can start as soon as the first half is transformed
            lrh = [
                [
                    constp.tile([128, M // 2], f16, name=f"lr{s}h{h}")
                    for h in range(2)
                ]
                for s in range(NSIDES)
            ]

            # ---- phase 0: inputs ----
            nc.sync.dma_start(wm[:], wm_d[:])
            # touch Sqrt now so its ACT table set loads during the prelude
            # (Copy lives in every set, so no second table switch later);
            # scale=0/bias=1 keeps the probe input in sqrt's valid range
            nc.scalar.activation(
                dummy[0:10, 0:1],
                wm[:, 0:1],
                mybir.ActivationFunctionType.Sqrt,
                bias=1.0,
                scale=0.0,
            )
            # chunked so the first transform matmul starts after 1/8 of F
            for q in range(NCH):
                nc.sync.dma_start(
                    F[:, q * 512 : (q + 1) * 512],
                    feat_d[:, q * 512 : (q + 1) * 512],
                )

            # ---- phase 1: L/R transforms (fp32 exact) ----
            # All four 5x10 transform matrices are stacked into one
            # [10, 128] lhsT (side s at columns 32s..32s+5): the fp32
            # matmul cost is per-row, independent of M, so one stacked
            # matmul per chunk does the work of four.  The [128, 512]
            # outputs are cast-evacuated to bf16 and the four 5-row
            # blocks DMA'd to partition 0 where the main matmuls expect
            # their K operands.
            # per-chunk evac + replica DMAs so phase 2 can start as soon as
            # the first half of the transforms lands
            with tc.tile_pool(name="pst", bufs=4, space="PSUM") as pst:
                for cc in range(NCH):
                    h, hc = cc // (NCH // 2), cc % (NCH // 2)
                    tp = pst.tile([128, 512], f32, tag="tp")
                    nc.tensor.matmul(tp[:], wm[:], F[:, cc * 512 : (cc + 1) * 512])
                    csl = slice(cc * 512, (cc + 1) * 512)
                    if cc % 2 == 0:
                        nc.scalar.copy(sbig[:, csl], tp[:])
                    else:
                        nc.vector.tensor_copy(sbig[:, csl], tp[:])
                    if hc == NCH // 2 - 1:
                        # replicate this half's four 5-row side blocks to the
                        # four 32-row offsets.  DMA *issue* costs ~800ns per
                        # descriptor on a sequencer, so spread the 16 DMAs
                        # over four otherwise-idle sequencers
                        hof = h * (M // 2)
                        emitters = [nc.sync, nc.gpsimd, nc.scalar, nc.gpsimd]
                        for s in range(NSIDES):
                            for i in range(4):
                                emitters[i].dma_start(
                                    lrh[s][h][32 * i : 32 * i + 5, :],
                                    sbig[32 * s : 32 * s + 5,
                                         hof : hof + M // 2],
                                )

            # ---- phase 2: pairwise d2 + min-reduce ----
            # HW allows only ONE PSUM operand per DVE op (and the custom
            # tensor_tensor_reduce ISA op is unavailable in this runtime),
            # so the d2 drain is split across standard ops: DVE
            # tensor_reduce(min) eats XDVE cols straight from PSUM (1x);
            # ScalarE evacuates the rest to fp16 SBUF where a 3-level
            # tensor_tensor(min) tree runs in the DVE 2x_1P mode, then a
            # final 1x reduce.  XDVE chosen so DVE and ACT are balanced.
            XDVE = 640
            E = 4096 - XDVE  # evacuated columns

            def emit_tree(eb, acc, col):
                l1 = ebp.tile([128, E // 2], f16, tag="l1")
                l2 = ebp.tile([128, E // 4], f16, tag="l2")
                l3 = ebp.tile([128, E // 8], f16, tag="l3")
                accb = accp.tile([128, 1], f32, tag="accb")
                nc.vector.tensor_tensor(
                    l1[:], eb[:, 0 : E // 2], eb[:, E // 2 : E], AL.min
                )
                nc.vector.tensor_tensor(
                    l2[:], l1[:, 0 : E // 4], l1[:, E // 4 : E // 2], AL.min
                )
                nc.vector.tensor_tensor(
                    l3[:], l2[:, 0 : E // 8], l2[:, E // 8 : E // 4], AL.min
                )
                nc.vector.tensor_reduce(
                    accb[:], l3[:], axis=mybir.AxisListType.X, op=AL.min
                )
                nc.vector.tensor_tensor(
                    mind2[:, col : col + 1], acc[:], accb[:], AL.min
                )

            # the fp16 tree of unit i is emitted during unit i+1 so the DVE
            # never stalls waiting for the ScalarE evacuation of its inputs
            with tc.tile_pool(name="psm", bufs=2, space="PSUM") as psm, \
                 tc.tile_pool(name="ebp", bufs=4) as ebp:
                pending = None
                for b in range(BPC):
                    for mt in range(NMT):
                        lb = lrh[2 * b][mt // (NMT // 2)]
                        rbh0 = lrh[2 * b + 1][0]
                        rbh1 = lrh[2 * b + 1][1]
                        mt0 = (mt % (NMT // 2)) * 128
                        mt1 = mt0 + 128
                        col = mt * BPC + b
                        acc = accp.tile([128, 1], f32, tag="acc")
                        eb = ebp.tile([128, E], f16, tag="eb")
                        pa = psm.tile([128, 2048], f32, tag="ps")
                        for c in range(4):
                            nc.tensor.matmul(
                                pa[:, c * 512 : (c + 1) * 512],
                                lb[32 * c : 32 * c + 5, mt0:mt1],
                                rbh0[32 * c : 32 * c + 5, c * 512 : (c + 1) * 512],
                                tile_position=(32 * c, 0),
                            )
                        nc.vector.tensor_reduce(
                            acc[:],
                            pa[:, 0:XDVE],
                            axis=mybir.AxisListType.X,
                            op=AL.min,
                        )
                        nc.scalar.copy(eb[:, 0 : 2048 - XDVE], pa[:, XDVE:2048])
                        pb = psm.tile([128, 2048], f32, tag="ps")
                        for c in range(4):
                            nc.tensor.matmul(
                                pb[:, c * 512 : (c + 1) * 512],
                                lb[32 * c : 32 * c + 5, mt0:mt1],
                                rbh1[32 * c : 32 * c + 5, c * 512 : (c + 1) * 512],
                                tile_position=(32 * c, 0),
                            )
                        nc.scalar.copy(eb[:, 2048 - XDVE : E], pb[:])
                        if pending is not None:
                            emit_tree(*pending)
                        pending = (eb, acc, col)
                emit_tree(*pending)

            # ---- phase 3: sqrt + reductions ----
            from concourse.bass_isa import ReduceOp

            nc.vector.tensor_scalar_max(dist[:], mind2[:], 0.0)
            nc.scalar.sqrt(mind2[:], dist[:])
            nc.vector.tensor_reduce(
                colsum[:], mind2[:], axis=mybir.AxisListType.X, op=AL.add
            )
            nc.gpsimd.partition_all_reduce(colsum[:], colsum[:], 128, ReduceOp.add)
            nc.sync.dma_start(out_d[:], colsum[0:1, :])

    nc.compile()
    return nc


def _host_inputs(pred_R, pred_t, gt_R, gt_t, model_points):
    # degree-2 polynomial feature map of the shared model points
    # (layout/feature prep of the constant cloud; all batch-dependent math
    # — transforms, distances, mins — runs on device)
    p = np.asarray(model_points, np.float32).T.astype(np.float32)  # [3, M]
    x, y, z = p[0], p[1], p[2]
    feat = np.stack(
        [x, y, z, np.ones_like(x), x * x, y * y, z * z, x * y, y * z, z * x]
    ).astype(np.float32)  # [10, M]
    in_maps = []
    for c in range(NCORES):
        wmat = np.zeros((10, 128), np.float32)
        for i in range(BPC):
            b = c * BPC + i
            wl = _w_pred(np.float64(pred_R[b]), np.float64(pred_t[b])).T
            wr = _w_gt(np.float64(gt_R[b]), np.float64(gt_t[b])).T
            wmat[:, 32 * (2 * i) : 32 * (2 * i) + 5] = wl
            wmat[:, 32 * (2 * i + 1) : 32 * (2 * i + 1) + 5] = wr
        in_maps.append({"feat": feat, "wmat": wmat})
    return in_maps


LAST_RESULT = None


def kernel(pred_R, pred_t, gt_R, gt_t, model_points):
    global LAST_RESULT
    from concourse.bass_utils import run_bass_kernel_spmd

    nc = _build_graph()
    in_maps = _host_inputs(pred_R, pred_t, gt_R, gt_t, model_points)
    res = run_bass_kernel_spmd(nc, in_maps, list(range(NCORES)))
    LAST_RESULT = res
    total = sum(float(res.results[c]["out"][0, 0]) for c in range(NCORES))
    return np.float32(total / (B * M))
